# revision 22
# baseline (speedup 1.0000x reference)
"""MultiLabelSupConLoss Trainium2 kernel (8-core SPMD, Bass/Tile).

Math
----
reference computes, with l_ij = <f0_i, f0_j>/T (f0 = features[:,0,:]):
    logits_max_i = max_j over the full [2B] row of contrast similarities
    e = exp(l[:B,:B] - logits_max)
    per_row = log(sum_j e_ij) - log(sum_{j in pos(i)} e_ij)
    loss = mean over rows with >=1 positive

per_row is invariant to ANY per-row shift c_i (it cancels in the
log-difference), so instead of the full-row max we use c_i = l_ii
(the self-similarity, which dominates every row by a huge margin for
normalized-random features; using it keeps exp() in range exactly like
the reference's row max does).  This removes the need to ever compute
the second half [B:2B] of the contrast matrix: those columns only
entered through logits_max.

The positive mask sim_ij >= 0.5 with sim = inter/(union+1e-6) is
equivalent (integer label counts) to z_ij = 3*inter - rs_i - rs_j >= 1,
computed by a single augmented matmul over K=102 (padded to 128):
    lhsT rows: [labels.T ; ones ; rs ; 0...],
    rhs rows:  [3*labels.T ; -rs ; -ones ; 0...]

Sharding: data-parallel over rows; each of the 8 cores handles 512 rows
and returns per-row (den, pos) partial sums; the host does the final
log/mean (a 4096-element epilogue).

Per core device pipeline, per (i-chunk 128 rows x jj-group 1024 cols):
    PE : l  = f0T_blk.T @ f0T       -> PSUM (2 matmuls, bf16 in, fp32 acc)
    PE : z  = labAug_blk.T @ labAug -> PSUM (2 matmuls)
    ACT: e  = exp(l + bias_i), accum_out -> den partial   (1 op, 1024 wide)
    DVE: (z >= 0.5) * e,      accum_out -> pos partial    (1 op, 1024 wide)
"""

import numpy as np
import ml_dtypes

import concourse.bass as bass
import concourse.bacc as bacc
import concourse.mybir as mybir
from concourse import tile
from concourse.bass_utils import run_bass_kernel_spmd

B = 4096
D = 128
N_CORES = 8
ROWS = B // N_CORES          # 512 rows per core
ICHUNK = 128                 # rows per i-chunk (PSUM partition dim)
IC = ROWS // ICHUNK          # 4
# column chunks: small first chunks so compute starts as soon as ~0.5MB
# of input has landed; 1024-wide steady chunks (2 PSUM banks)
CHUNKS = [512, 512, 1024, 1024, 1024]
NCH = len(CHUNKS)
CH_OFF = [sum(CHUNKS[:i]) for i in range(NCH)]
KLAB = 128                   # 100 label dims + 2 augmentation rows + pad
TEMP = 0.07

BF16 = ml_dtypes.bfloat16

_cached = None


def _build_nc():
    f32 = mybir.dt.float32
    bf16 = mybir.dt.bfloat16
    nc = bacc.Bacc(
        "TRN2",
        target_bir_lowering=False,
        debug=False,
        num_devices=N_CORES,
    )

    fT_d = nc.dram_tensor("ft_full", [D, B], bf16, kind="ExternalInput")
    fTb_d = nc.dram_tensor("ft_blk", [D, ROWS], bf16, kind="ExternalInput")
    labR_d = nc.dram_tensor("lab_full", [KLAB, B], bf16, kind="ExternalInput")
    labL_d = nc.dram_tensor("lab_blk", [KLAB, ROWS], bf16, kind="ExternalInput")
    bias_d = nc.dram_tensor("bias", [ICHUNK, IC], f32, kind="ExternalInput")
    den_d = nc.dram_tensor("den", [ICHUNK, IC * NCH], f32, kind="ExternalOutput")
    pos_d = nc.dram_tensor("pos", [ICHUNK, IC * NCH], f32, kind="ExternalOutput")

    act_exp = mybir.ActivationFunctionType.Exp

    with tile.TileContext(nc) as tc:
        with (
            tc.tile_pool(name="const", bufs=1) as cpool,
            tc.tile_pool(name="e", bufs=3) as epool,
            tc.tile_pool(name="em", bufs=2) as empool,
            tc.tile_pool(name="psl", bufs=2, space="PSUM") as psl,
            tc.tile_pool(name="psz", bufs=2, space="PSUM") as psz,
        ):
            fT_s = cpool.tile([D, B], bf16)
            fTb_s = cpool.tile([D, ROWS], bf16)
            labR_s = cpool.tile([KLAB, B], bf16)
            labL_s = cpool.tile([KLAB, ROWS], bf16)
            bias_s = cpool.tile([ICHUNK, IC], f32)
            den_s = cpool.tile([ICHUNK, IC * NCH], f32)
            pos_s = cpool.tile([ICHUNK, IC * NCH], f32)
            scratch = cpool.tile([1, 8], f32)

            # Small, first-needed operands spread across both HWDGE rings
            # (scalar/ACT + sync/SP) and the gpsimd SWDGE path; big tensors
            # split by column chunk so compute starts after the first
            # ~0.5MB instead of after the full 2MB.
            # Spread loads over all three DMA paths, in need order. The SP
            # ring (sync) is the fastest and carries the pipeline-critical
            # early chunks; the ACT ring and SWDGE (gpsimd) carry small /
            # late chunks whose higher latency is hidden behind compute.
            def _ch(ch):
                return slice(CH_OFF[ch], CH_OFF[ch] + CHUNKS[ch])

            nc.scalar.dma_start(fTb_s[:], fTb_d[:])
            nc.scalar.dma_start(bias_s[:], bias_d[:])
            nc.sync.dma_start(labL_s[:], labL_d[:])
            nc.sync.dma_start(labR_s[:, _ch(0)], labR_d[:, _ch(0)])
            nc.sync.dma_start(fT_s[:, _ch(0)], fT_d[:, _ch(0)])
            nc.sync.dma_start(fT_s[:, _ch(1)], fT_d[:, _ch(1)])
            nc.sync.dma_start(labR_s[:, _ch(1)], labR_d[:, _ch(1)])
            nc.sync.dma_start(fT_s[:, _ch(2)], fT_d[:, _ch(2)])
            nc.sync.dma_start(labR_s[:, _ch(2)], labR_d[:, _ch(2)])
            nc.sync.dma_start(fT_s[:, _ch(3)], fT_d[:, _ch(3)])
            nc.sync.dma_start(labR_s[:, _ch(3)], labR_d[:, _ch(3)])
            nc.sync.dma_start(fT_s[:, _ch(4)], fT_d[:, _ch(4)])
            nc.gpsimd.dma_start(labR_s[:, _ch(4)], labR_d[:, _ch(4)])

            # pre-load the exp spline tables while input DMAs stream
            nc.vector.memset(scratch[:], 0.0)
            nc.scalar.activation(
                scratch[:], scratch[:], act_exp, bias=scratch[:, 0:1]
            )

            # column-chunk outer, row-chunk inner: only chunk 0 gates the
            # first matmul; later chunks stream in behind compute.
            for ch in range(NCH):
                w = CHUNKS[ch]
                nmm = w // 512
                for ic in range(IC):
                    isl = slice(ic * ICHUNK, (ic + 1) * ICHUNK)
                    col = ic * NCH + ch

                    l_ps = psl.tile([ICHUNK, w], f32)
                    z_ps = psz.tile([ICHUNK, w], f32)
                    for h in range(nmm):
                        jsl = slice(CH_OFF[ch] + h * 512, CH_OFF[ch] + (h + 1) * 512)
                        hsl = slice(h * 512, (h + 1) * 512)
                        nc.tensor.matmul(z_ps[:, hsl], labL_s[:, isl], labR_s[:, jsl])
                        nc.tensor.matmul(l_ps[:, hsl], fTb_s[:, isl], fT_s[:, jsl])

                    e_t = epool.tile([ICHUNK, w], f32, tag="e")
                    nc.scalar.activation(
                        e_t[:],
                        l_ps[:],
                        act_exp,
                        bias=bias_s[:, ic : ic + 1],
                        scale=1.0,
                        accum_out=den_s[:, col : col + 1],
                    )

                    em_t = empool.tile([ICHUNK, w], bf16, tag="em")
                    nc.vector.scalar_tensor_tensor(
                        em_t[:],
                        z_ps[:],
                        0.5,
                        e_t[:],
                        op0=mybir.AluOpType.is_ge,
                        op1=mybir.AluOpType.mult,
                        accum_out=pos_s[:, col : col + 1],
                    )

            # den completes with the last exp (before the last stt): ship it
            # early; pos right after its last accumulation. Host folds the
            # NCH chunk partials per row.
            nc.scalar.dma_start(den_d[:], den_s[:])
            nc.sync.dma_start(pos_d[:], pos_s[:])

    nc.compile()
    names = {
        "fT": fT_d.name,
        "fTb": fTb_d.name,
        "labR": labR_d.name,
        "labL": labL_d.name,
        "bias": bias_d.name,
        "den": den_d.name,
        "pos": pos_d.name,
    }
    return nc, names


def _get_nc():
    global _cached
    if _cached is None:
        _cached = _build_nc()
    return _cached


def _prep_inputs(features, labels):
    """Host-side shard prep: transposed/casted operand layouts per core."""
    f0 = np.asarray(features)[:, 0, :].astype(np.float32)      # [B, D]
    lab = np.asarray(labels).astype(np.float32)                # [B, 100]

    s = np.float32(1.0) / np.float32(np.sqrt(np.float32(TEMP)))
    fT16 = np.ascontiguousarray((f0 * s).T).astype(BF16)       # [D, B] bf16
    # row self-similarity (= diagonal of l), from the same bf16 values
    c = (fT16.astype(np.float32) ** 2).sum(axis=0, dtype=np.float32)  # [B]

    rs = lab.sum(axis=1, dtype=np.float32)                     # [B] integers
    labT = lab.T                                               # [100, B]
    L = np.zeros((KLAB, B), dtype=np.float32)
    L[:100] = labT
    L[100] = 1.0
    L[101] = rs
    R = np.zeros((KLAB, B), dtype=np.float32)
    R[:100] = 3.0 * labT
    R[100] = -rs
    R[101] = -1.0
    L16 = L.astype(BF16)
    R16 = R.astype(BF16)

    nc, names = _get_nc()
    in_maps = []
    for core in range(N_CORES):
        blk = slice(core * ROWS, (core + 1) * ROWS)
        bias = np.ascontiguousarray(
            (-c[blk]).reshape(IC, ICHUNK).T.astype(np.float32)
        )
        in_maps.append(
            {
                names["fT"]: fT16,
                names["fTb"]: np.ascontiguousarray(fT16[:, blk]),
                names["labR"]: R16,
                names["labL"]: np.ascontiguousarray(L16[:, blk]),
                names["bias"]: bias,
            }
        )
    return nc, names, in_maps


def _finish(results, names):
    """Host epilogue: per-row log-ratio + masked mean over 4096 rows."""
    den = np.empty(B, dtype=np.float32)
    pos = np.empty(B, dtype=np.float32)
    for core, r in enumerate(results):
        blk = slice(core * ROWS, (core + 1) * ROWS)
        # [128, IC*NCH] chunk partials -> [128, IC] row sums -> row order
        dc = r[names["den"]].reshape(ICHUNK, IC, NCH).sum(axis=2, dtype=np.float32)
        pc = r[names["pos"]].reshape(ICHUNK, IC, NCH).sum(axis=2, dtype=np.float32)
        den[blk] = dc.T.reshape(ROWS)
        pos[blk] = pc.T.reshape(ROWS)
    has = pos > 0
    per_row = np.zeros(B, dtype=np.float32)
    per_row[has] = np.log(den[has]) - np.log(pos[has])
    count = np.float32(max(int(has.sum()), 1))
    loss = np.float32(per_row.sum(dtype=np.float32) / count)
    return np.asarray(loss, dtype=np.float32)


def kernel(features, labels):
    nc, names, in_maps = _prep_inputs(features, labels)
    res = run_bass_kernel_spmd(nc, in_maps, list(range(N_CORES)))
    return _finish(res.results, names)


def kernel_with_results(features, labels, **spmd_kwargs):
    """Like kernel() but also returns the BassKernelResults (for tracing)."""
    nc, names, in_maps = _prep_inputs(features, labels)
    res = run_bass_kernel_spmd(nc, in_maps, list(range(N_CORES)), **spmd_kwargs)
    return _finish(res.results, names), res


# revision 23
# speedup vs baseline: 1.0545x; 1.0545x over previous
"""MultiLabelSupConLoss Trainium2 kernel (8-core SPMD, Bass/Tile).

Math
----
reference computes, with l_ij = <f0_i, f0_j>/T (f0 = features[:,0,:]):
    logits_max_i = max_j over the full [2B] row of contrast similarities
    e = exp(l[:B,:B] - logits_max)
    per_row = log(sum_j e_ij) - log(sum_{j in pos(i)} e_ij)
    loss = mean over rows with >=1 positive

per_row is invariant to ANY per-row shift c_i (it cancels in the
log-difference), so instead of the full-row max we use c_i = l_ii
(the self-similarity, which dominates every row by a huge margin for
normalized-random features; using it keeps exp() in range exactly like
the reference's row max does).  This removes the need to ever compute
the second half [B:2B] of the contrast matrix: those columns only
entered through logits_max.

The positive mask sim_ij >= 0.5 with sim = inter/(union+1e-6) is
equivalent (integer label counts) to z_ij = 3*inter - rs_i - rs_j >= 1,
computed by a single augmented matmul over K=102 (padded to 128):
    lhsT rows: [labels.T ; ones ; rs ; 0...],
    rhs rows:  [3*labels.T ; -rs ; -ones ; 0...]

Sharding: data-parallel over rows; each of the 8 cores handles 512 rows
and returns per-row (den, pos) partial sums; the host does the final
log/mean (a 4096-element epilogue).

Per core device pipeline, per (i-chunk 128 rows x jj-group 1024 cols):
    PE : l  = f0T_blk.T @ f0T       -> PSUM (2 matmuls, bf16 in, fp32 acc)
    PE : z  = labAug_blk.T @ labAug -> PSUM (2 matmuls)
    ACT: e  = exp(l + bias_i), accum_out -> den partial   (1 op, 1024 wide)
    DVE: (z >= 0.5) * e,      accum_out -> pos partial    (1 op, 1024 wide)
"""

import numpy as np
import ml_dtypes

import concourse.bass as bass
import concourse.bacc as bacc
import concourse.mybir as mybir
from concourse import tile
from concourse.bass_utils import run_bass_kernel_spmd

B = 4096
D = 128
N_CORES = 8
ROWS = B // N_CORES          # 512 rows per core
ICHUNK = 128                 # rows per i-chunk (PSUM partition dim)
IC = ROWS // ICHUNK          # 4
# column chunks: small first chunks so compute starts as soon as ~0.5MB
# of input has landed; 1024-wide steady chunks (2 PSUM banks)
CHUNKS = [512, 512, 1024, 1024, 1024]
NCH = len(CHUNKS)
CH_OFF = [sum(CHUNKS[:i]) for i in range(NCH)]
KLAB = 128                   # 100 label dims + 2 augmentation rows + pad
TEMP = 0.07

BF16 = ml_dtypes.bfloat16

_cached = None


def _build_nc():
    f32 = mybir.dt.float32
    bf16 = mybir.dt.bfloat16
    nc = bacc.Bacc(
        "TRN2",
        target_bir_lowering=False,
        debug=False,
        num_devices=N_CORES,
    )

    fT_d = nc.dram_tensor("ft_full", [D, B], bf16, kind="ExternalInput")
    fTb_d = nc.dram_tensor("ft_blk", [D, ROWS], bf16, kind="ExternalInput")
    labR_d = nc.dram_tensor("lab_full", [KLAB, B], bf16, kind="ExternalInput")
    labL_d = nc.dram_tensor("lab_blk", [KLAB, ROWS], bf16, kind="ExternalInput")
    bias_d = nc.dram_tensor("bias", [ICHUNK, IC], f32, kind="ExternalInput")
    den_d = nc.dram_tensor("den", [ICHUNK, IC * NCH], f32, kind="ExternalOutput")
    pos_d = nc.dram_tensor("pos", [ICHUNK, IC * NCH], f32, kind="ExternalOutput")

    act_exp = mybir.ActivationFunctionType.Exp

    with tile.TileContext(nc) as tc:
        with (
            tc.tile_pool(name="const", bufs=1) as cpool,
            tc.tile_pool(name="e", bufs=3) as epool,
            tc.tile_pool(name="em", bufs=2) as empool,
            tc.tile_pool(name="psl", bufs=2, space="PSUM") as psl,
            tc.tile_pool(name="psz", bufs=2, space="PSUM") as psz,
        ):
            fT_s = cpool.tile([D, B], bf16)
            fTb_s = cpool.tile([D, ROWS], bf16)
            labR_s = cpool.tile([KLAB, B], bf16)
            labL_s = cpool.tile([KLAB, ROWS], bf16)
            bias_s = cpool.tile([ICHUNK, IC], f32)
            den_s = cpool.tile([ICHUNK, IC * NCH], f32)
            pos_s = cpool.tile([ICHUNK, IC * NCH], f32)
            scratch = cpool.tile([1, 8], f32)

            # Small, first-needed operands spread across both HWDGE rings
            # (scalar/ACT + sync/SP) and the gpsimd SWDGE path; big tensors
            # split by column chunk so compute starts after the first
            # ~0.5MB instead of after the full 2MB.
            # Spread loads over all three DMA paths, in need order. The SP
            # ring (sync) is the fastest and carries the pipeline-critical
            # early chunks; the ACT ring and SWDGE (gpsimd) carry small /
            # late chunks whose higher latency is hidden behind compute.
            def _ch(ch):
                return slice(CH_OFF[ch], CH_OFF[ch] + CHUNKS[ch])

            nc.scalar.dma_start(fTb_s[:], fTb_d[:])
            nc.scalar.dma_start(bias_s[:], bias_d[:])
            nc.sync.dma_start(labL_s[:], labL_d[:])
            nc.sync.dma_start(labR_s[:, _ch(0)], labR_d[:, _ch(0)])
            nc.sync.dma_start(fT_s[:, _ch(0)], fT_d[:, _ch(0)])
            nc.sync.dma_start(fT_s[:, _ch(1)], fT_d[:, _ch(1)])
            nc.sync.dma_start(labR_s[:, _ch(1)], labR_d[:, _ch(1)])
            nc.sync.dma_start(fT_s[:, _ch(2)], fT_d[:, _ch(2)])
            nc.sync.dma_start(labR_s[:, _ch(2)], labR_d[:, _ch(2)])
            nc.sync.dma_start(fT_s[:, _ch(3)], fT_d[:, _ch(3)])
            nc.sync.dma_start(labR_s[:, _ch(3)], labR_d[:, _ch(3)])
            nc.sync.dma_start(fT_s[:, _ch(4)], fT_d[:, _ch(4)])
            nc.sync.dma_start(labR_s[:, _ch(4)], labR_d[:, _ch(4)])

            # pre-load the exp spline tables while input DMAs stream
            nc.vector.memset(scratch[:], 0.0)
            nc.scalar.activation(
                scratch[:], scratch[:], act_exp, bias=scratch[:, 0:1]
            )

            # column-chunk outer, row-chunk inner: only chunk 0 gates the
            # first matmul; later chunks stream in behind compute.
            for ch in range(NCH):
                w = CHUNKS[ch]
                nmm = w // 512
                for ic in range(IC):
                    isl = slice(ic * ICHUNK, (ic + 1) * ICHUNK)
                    col = ic * NCH + ch

                    l_ps = psl.tile([ICHUNK, w], f32)
                    z_ps = psz.tile([ICHUNK, w], f32)
                    for h in range(nmm):
                        jsl = slice(CH_OFF[ch] + h * 512, CH_OFF[ch] + (h + 1) * 512)
                        hsl = slice(h * 512, (h + 1) * 512)
                        nc.tensor.matmul(z_ps[:, hsl], labL_s[:, isl], labR_s[:, jsl])
                        nc.tensor.matmul(l_ps[:, hsl], fTb_s[:, isl], fT_s[:, jsl])

                    e_t = epool.tile([ICHUNK, w], f32, tag="e")
                    nc.scalar.activation(
                        e_t[:],
                        l_ps[:],
                        act_exp,
                        bias=bias_s[:, ic : ic + 1],
                        scale=1.0,
                        accum_out=den_s[:, col : col + 1],
                    )

                    em_t = empool.tile([ICHUNK, w], bf16, tag="em")
                    nc.vector.scalar_tensor_tensor(
                        em_t[:],
                        z_ps[:],
                        0.5,
                        e_t[:],
                        op0=mybir.AluOpType.is_ge,
                        op1=mybir.AluOpType.mult,
                        accum_out=pos_s[:, col : col + 1],
                    )

            # den completes with the last exp (before the last stt): ship it
            # early; pos right after its last accumulation. Host folds the
            # NCH chunk partials per row.
            nc.scalar.dma_start(den_d[:], den_s[:])
            nc.sync.dma_start(pos_d[:], pos_s[:])

    nc.compile()
    names = {
        "fT": fT_d.name,
        "fTb": fTb_d.name,
        "labR": labR_d.name,
        "labL": labL_d.name,
        "bias": bias_d.name,
        "den": den_d.name,
        "pos": pos_d.name,
    }
    return nc, names


def _get_nc():
    global _cached
    if _cached is None:
        _cached = _build_nc()
    return _cached


def _prep_inputs(features, labels):
    """Host-side shard prep: transposed/casted operand layouts per core."""
    f0 = np.asarray(features)[:, 0, :].astype(np.float32)      # [B, D]
    lab = np.asarray(labels).astype(np.float32)                # [B, 100]

    s = np.float32(1.0) / np.float32(np.sqrt(np.float32(TEMP)))
    fT16 = np.ascontiguousarray((f0 * s).T).astype(BF16)       # [D, B] bf16
    # row self-similarity (= diagonal of l), from the same bf16 values
    c = (fT16.astype(np.float32) ** 2).sum(axis=0, dtype=np.float32)  # [B]

    rs = lab.sum(axis=1, dtype=np.float32)                     # [B] integers
    labT = lab.T                                               # [100, B]
    L = np.zeros((KLAB, B), dtype=np.float32)
    L[:100] = labT
    L[100] = 1.0
    L[101] = rs
    R = np.zeros((KLAB, B), dtype=np.float32)
    R[:100] = 3.0 * labT
    R[100] = -rs
    R[101] = -1.0
    L16 = L.astype(BF16)
    R16 = R.astype(BF16)

    nc, names = _get_nc()
    in_maps = []
    for core in range(N_CORES):
        blk = slice(core * ROWS, (core + 1) * ROWS)
        bias = np.ascontiguousarray(
            (-c[blk]).reshape(IC, ICHUNK).T.astype(np.float32)
        )
        in_maps.append(
            {
                names["fT"]: fT16,
                names["fTb"]: np.ascontiguousarray(fT16[:, blk]),
                names["labR"]: R16,
                names["labL"]: np.ascontiguousarray(L16[:, blk]),
                names["bias"]: bias,
            }
        )
    return nc, names, in_maps


def _finish(results, names):
    """Host epilogue: per-row log-ratio + masked mean over 4096 rows."""
    den = np.empty(B, dtype=np.float32)
    pos = np.empty(B, dtype=np.float32)
    for core, r in enumerate(results):
        blk = slice(core * ROWS, (core + 1) * ROWS)
        # [128, IC*NCH] chunk partials -> [128, IC] row sums -> row order
        dc = r[names["den"]].reshape(ICHUNK, IC, NCH).sum(axis=2, dtype=np.float32)
        pc = r[names["pos"]].reshape(ICHUNK, IC, NCH).sum(axis=2, dtype=np.float32)
        den[blk] = dc.T.reshape(ROWS)
        pos[blk] = pc.T.reshape(ROWS)
    has = pos > 0
    per_row = np.zeros(B, dtype=np.float32)
    per_row[has] = np.log(den[has]) - np.log(pos[has])
    count = np.float32(max(int(has.sum()), 1))
    loss = np.float32(per_row.sum(dtype=np.float32) / count)
    return np.asarray(loss, dtype=np.float32)


def kernel(features, labels):
    nc, names, in_maps = _prep_inputs(features, labels)
    res = run_bass_kernel_spmd(nc, in_maps, list(range(N_CORES)))
    return _finish(res.results, names)


def kernel_with_results(features, labels, **spmd_kwargs):
    """Like kernel() but also returns the BassKernelResults (for tracing)."""
    nc, names, in_maps = _prep_inputs(features, labels)
    res = run_bass_kernel_spmd(nc, in_maps, list(range(N_CORES)), **spmd_kwargs)
    return _finish(res.results, names), res


# revision 24
# speedup vs baseline: 1.0988x; 1.0421x over previous
"""MultiLabelSupConLoss Trainium2 kernel (8-core SPMD, Bass/Tile).

Math
----
reference computes, with l_ij = <f0_i, f0_j>/T (f0 = features[:,0,:]):
    logits_max_i = max_j over the full [2B] row of contrast similarities
    e = exp(l[:B,:B] - logits_max)
    per_row = log(sum_j e_ij) - log(sum_{j in pos(i)} e_ij)
    loss = mean over rows with >=1 positive

per_row is invariant to ANY per-row shift c_i (it cancels in the
log-difference), so instead of the full-row max we use c_i = l_ii
(the self-similarity, which dominates every row by a huge margin for
normalized-random features; using it keeps exp() in range exactly like
the reference's row max does).  This removes the need to ever compute
the second half [B:2B] of the contrast matrix: those columns only
entered through logits_max.

The positive mask sim_ij >= 0.5 with sim = inter/(union+1e-6) is
equivalent (integer label counts) to z_ij = 3*inter - rs_i - rs_j >= 1,
computed by a single augmented matmul over K=102 (padded to 128):
    lhsT rows: [labels.T ; ones ; rs ; 0...],
    rhs rows:  [3*labels.T ; -rs ; -ones ; 0...]

Sharding: data-parallel over rows; each of the 8 cores handles 512 rows
and returns per-row (den, pos) partial sums; the host does the final
log/mean (a 4096-element epilogue).

Per core device pipeline, per (i-chunk 128 rows x jj-group 1024 cols):
    PE : l  = f0T_blk.T @ f0T       -> PSUM (2 matmuls, bf16 in, fp32 acc)
    PE : z  = labAug_blk.T @ labAug -> PSUM (2 matmuls)
    ACT: e  = exp(l + bias_i), accum_out -> den partial   (1 op, 1024 wide)
    DVE: (z >= 0.5) * e,      accum_out -> pos partial    (1 op, 1024 wide)
"""

import numpy as np
import ml_dtypes

import concourse.bass as bass
import concourse.bacc as bacc
import concourse.mybir as mybir
from concourse import tile
from concourse.bass_utils import run_bass_kernel_spmd

B = 4096
D = 128
N_CORES = 8
ROWS = B // N_CORES          # 512 rows per core
ICHUNK = 128                 # rows per i-chunk (PSUM partition dim)
IC = ROWS // ICHUNK          # 4
# column chunks: small first chunks so compute starts as soon as ~0.5MB
# of input has landed; 1024-wide steady chunks (2 PSUM banks)
CHUNKS = [512, 512, 1024, 1024, 1024]
NCH = len(CHUNKS)
CH_OFF = [sum(CHUNKS[:i]) for i in range(NCH)]
KLAB = 128                   # 100 label dims + 2 augmentation rows + pad
TEMP = 0.07

BF16 = ml_dtypes.bfloat16

_cached = None


def _build_nc():
    f32 = mybir.dt.float32
    bf16 = mybir.dt.bfloat16
    nc = bacc.Bacc(
        "TRN2",
        target_bir_lowering=False,
        debug=False,
        num_devices=N_CORES,
    )

    fT_d = nc.dram_tensor("ft_full", [D, B], bf16, kind="ExternalInput")
    fTb_d = nc.dram_tensor("ft_blk", [D, ROWS], bf16, kind="ExternalInput")
    labR_d = nc.dram_tensor("lab_full", [KLAB, B], bf16, kind="ExternalInput")
    labL_d = nc.dram_tensor("lab_blk", [KLAB, ROWS], bf16, kind="ExternalInput")
    bias_d = nc.dram_tensor("bias", [ICHUNK, IC], f32, kind="ExternalInput")
    den_d = nc.dram_tensor("den", [ICHUNK, IC * NCH], f32, kind="ExternalOutput")
    pos_d = nc.dram_tensor("pos", [ICHUNK, IC * NCH], f32, kind="ExternalOutput")

    act_exp = mybir.ActivationFunctionType.Exp

    with tile.TileContext(nc) as tc:
        with (
            tc.tile_pool(name="const", bufs=1) as cpool,
            tc.tile_pool(name="e", bufs=4) as epool,
            tc.tile_pool(name="em", bufs=3) as empool,
            tc.tile_pool(name="psl", bufs=2, space="PSUM") as psl,
            tc.tile_pool(name="psz", bufs=2, space="PSUM") as psz,
        ):
            fT_s = cpool.tile([D, B], bf16)
            fTb_s = cpool.tile([D, ROWS], bf16)
            labR_s = cpool.tile([KLAB, B], bf16)
            labL_s = cpool.tile([KLAB, ROWS], bf16)
            bias_s = cpool.tile([ICHUNK, IC], f32)
            den_s = cpool.tile([ICHUNK, IC * NCH], f32)
            pos_s = cpool.tile([ICHUNK, IC * NCH], f32)
            scratch = cpool.tile([1, 8], f32)

            # Small, first-needed operands spread across both HWDGE rings
            # (scalar/ACT + sync/SP) and the gpsimd SWDGE path; big tensors
            # split by column chunk so compute starts after the first
            # ~0.5MB instead of after the full 2MB.
            # Spread loads over all three DMA paths, in need order. The SP
            # ring (sync) is the fastest and carries the pipeline-critical
            # early chunks; the ACT ring and SWDGE (gpsimd) carry small /
            # late chunks whose higher latency is hidden behind compute.
            def _ch(ch):
                return slice(CH_OFF[ch], CH_OFF[ch] + CHUNKS[ch])

            nc.scalar.dma_start(fTb_s[:], fTb_d[:])
            nc.scalar.dma_start(bias_s[:], bias_d[:])
            nc.sync.dma_start(labL_s[:], labL_d[:])
            nc.sync.dma_start(labR_s[:, _ch(0)], labR_d[:, _ch(0)])
            nc.sync.dma_start(fT_s[:, _ch(0)], fT_d[:, _ch(0)])
            nc.sync.dma_start(fT_s[:, _ch(1)], fT_d[:, _ch(1)])
            nc.sync.dma_start(labR_s[:, _ch(1)], labR_d[:, _ch(1)])
            nc.sync.dma_start(fT_s[:, _ch(2)], fT_d[:, _ch(2)])
            nc.sync.dma_start(labR_s[:, _ch(2)], labR_d[:, _ch(2)])
            nc.sync.dma_start(fT_s[:, _ch(3)], fT_d[:, _ch(3)])
            nc.sync.dma_start(labR_s[:, _ch(3)], labR_d[:, _ch(3)])
            nc.sync.dma_start(fT_s[:, _ch(4)], fT_d[:, _ch(4)])
            nc.sync.dma_start(labR_s[:, _ch(4)], labR_d[:, _ch(4)])

            # pre-load the exp spline tables while input DMAs stream
            nc.vector.memset(scratch[:], 0.0)
            nc.scalar.activation(
                scratch[:], scratch[:], act_exp, bias=scratch[:, 0:1]
            )

            # column-chunk outer, row-chunk inner: only chunk 0 gates the
            # first matmul; later chunks stream in behind compute.
            for ch in range(NCH):
                w = CHUNKS[ch]
                nmm = w // 512
                for ic in range(IC):
                    isl = slice(ic * ICHUNK, (ic + 1) * ICHUNK)
                    col = ic * NCH + ch

                    l_ps = psl.tile([ICHUNK, w], f32)
                    z_ps = psz.tile([ICHUNK, w], f32)
                    for h in range(nmm):
                        jsl = slice(CH_OFF[ch] + h * 512, CH_OFF[ch] + (h + 1) * 512)
                        hsl = slice(h * 512, (h + 1) * 512)
                        nc.tensor.matmul(z_ps[:, hsl], labL_s[:, isl], labR_s[:, jsl])
                        nc.tensor.matmul(l_ps[:, hsl], fTb_s[:, isl], fT_s[:, jsl])

                    e_t = epool.tile([ICHUNK, w], f32, tag="e")
                    nc.scalar.activation(
                        e_t[:],
                        l_ps[:],
                        act_exp,
                        bias=bias_s[:, ic : ic + 1],
                        scale=1.0,
                        accum_out=den_s[:, col : col + 1],
                    )

                    em_t = empool.tile([ICHUNK, w], bf16, tag="em")
                    nc.vector.scalar_tensor_tensor(
                        em_t[:],
                        z_ps[:],
                        0.5,
                        e_t[:],
                        op0=mybir.AluOpType.is_ge,
                        op1=mybir.AluOpType.mult,
                        accum_out=pos_s[:, col : col + 1],
                    )

            # den completes with the last exp (before the last stt): ship it
            # early; pos right after its last accumulation. Host folds the
            # NCH chunk partials per row.
            nc.scalar.dma_start(den_d[:], den_s[:])
            nc.sync.dma_start(pos_d[:], pos_s[:])

    nc.compile()
    names = {
        "fT": fT_d.name,
        "fTb": fTb_d.name,
        "labR": labR_d.name,
        "labL": labL_d.name,
        "bias": bias_d.name,
        "den": den_d.name,
        "pos": pos_d.name,
    }
    return nc, names


def _get_nc():
    global _cached
    if _cached is None:
        _cached = _build_nc()
    return _cached


def _prep_inputs(features, labels):
    """Host-side shard prep: transposed/casted operand layouts per core."""
    f0 = np.asarray(features)[:, 0, :].astype(np.float32)      # [B, D]
    lab = np.asarray(labels).astype(np.float32)                # [B, 100]

    s = np.float32(1.0) / np.float32(np.sqrt(np.float32(TEMP)))
    fT16 = np.ascontiguousarray((f0 * s).T).astype(BF16)       # [D, B] bf16
    # row self-similarity (= diagonal of l), from the same bf16 values
    c = (fT16.astype(np.float32) ** 2).sum(axis=0, dtype=np.float32)  # [B]

    rs = lab.sum(axis=1, dtype=np.float32)                     # [B] integers
    labT = lab.T                                               # [100, B]
    L = np.zeros((KLAB, B), dtype=np.float32)
    L[:100] = labT
    L[100] = 1.0
    L[101] = rs
    R = np.zeros((KLAB, B), dtype=np.float32)
    R[:100] = 3.0 * labT
    R[100] = -rs
    R[101] = -1.0
    L16 = L.astype(BF16)
    R16 = R.astype(BF16)

    nc, names = _get_nc()
    in_maps = []
    for core in range(N_CORES):
        blk = slice(core * ROWS, (core + 1) * ROWS)
        bias = np.ascontiguousarray(
            (-c[blk]).reshape(IC, ICHUNK).T.astype(np.float32)
        )
        in_maps.append(
            {
                names["fT"]: fT16,
                names["fTb"]: np.ascontiguousarray(fT16[:, blk]),
                names["labR"]: R16,
                names["labL"]: np.ascontiguousarray(L16[:, blk]),
                names["bias"]: bias,
            }
        )
    return nc, names, in_maps


def _finish(results, names):
    """Host epilogue: per-row log-ratio + masked mean over 4096 rows."""
    den = np.empty(B, dtype=np.float32)
    pos = np.empty(B, dtype=np.float32)
    for core, r in enumerate(results):
        blk = slice(core * ROWS, (core + 1) * ROWS)
        # [128, IC*NCH] chunk partials -> [128, IC] row sums -> row order
        dc = r[names["den"]].reshape(ICHUNK, IC, NCH).sum(axis=2, dtype=np.float32)
        pc = r[names["pos"]].reshape(ICHUNK, IC, NCH).sum(axis=2, dtype=np.float32)
        den[blk] = dc.T.reshape(ROWS)
        pos[blk] = pc.T.reshape(ROWS)
    has = pos > 0
    per_row = np.zeros(B, dtype=np.float32)
    per_row[has] = np.log(den[has]) - np.log(pos[has])
    count = np.float32(max(int(has.sum()), 1))
    loss = np.float32(per_row.sum(dtype=np.float32) / count)
    return np.asarray(loss, dtype=np.float32)


def kernel(features, labels):
    nc, names, in_maps = _prep_inputs(features, labels)
    res = run_bass_kernel_spmd(nc, in_maps, list(range(N_CORES)))
    return _finish(res.results, names)


def kernel_with_results(features, labels, **spmd_kwargs):
    """Like kernel() but also returns the BassKernelResults (for tracing)."""
    nc, names, in_maps = _prep_inputs(features, labels)
    res = run_bass_kernel_spmd(nc, in_maps, list(range(N_CORES)), **spmd_kwargs)
    return _finish(res.results, names), res
